# revision 30
# baseline (speedup 1.0000x reference)
"""Causal multi-head attention (B=4, T=2048, D=1024, H=16) on 8 TRN2 NeuronCores.

Sharding: core c -> batch b = c // 2, head-group g = c % 2 (8 heads each).
Host pre-transposes x to x^T per batch, converts everything to bf16, and
pre-slices W_qkv/W_o/biases per head-group (1/sqrt(dh)=1/8 folded into
W_q/b_q exactly).  Each core:

  phase 1 (two T-halves, interleaved with attention):
      Q^T,K^T  (qkv^T layout, heads pair-stacked on partitions); V natural,
      bf16 matmuls with dual-psum loop order so each stationary loads once.
      V bias added on DVE via a broadcast tile (no bias matmul).
  phase 2: attention, q-chunk-outer / head-pair-inner so chunks 0,1 (which
      only need the first T/2 of K/V/Q) start right after t-half 0; after
      t-half 1 the big chunk 3 runs BEFORE chunk 2 so chunk 3's oproj hides
      under chunk 2's attention and only the small chunk-2 tail remains.
      k-tiles processed in PAIRS (S,S -> exp,exp -> AV,AV) to halve the
      fill/drain transition count on the PE.
      exp engines balanced per chunk: ScalarE exp (+ GpSimd trimask mult on
      diagonal tiles) where ACT has slack (early chunks), DVE fused
      Schraudolph exp+mask (int16-saturation -> bf16 -0.0) where ACT is the
      limiter (late chunks).
      Row-sum reciprocals batched per chunk: raw [1,512] rows DMA to DRAM,
      reload respread as [128,32], one reciprocal, broadcast back.
  phase 3: output projection per q-chunk, trailing attention by one chunk.

Host sums the two head-group partials per batch and adds b_o.
"""

import sys

sys.path.insert(0, "/opt/trn_rl_repo")

import numpy as np

import concourse.bass as bass
import concourse.mybir as mybir
from concourse.bass_utils import run_bass_kernel_spmd
from concourse.tile import TileContext

F32 = mybir.dt.float32
BF16 = mybir.dt.bfloat16
I16 = mybir.dt.int16
EXP = mybir.ActivationFunctionType.Exp
COPY = mybir.ActivationFunctionType.Copy
MULT = mybir.AluOpType.mult
ADD = mybir.AluOpType.add

B, T, D, H = 4, 2048, 1024, 16
DH = D // H          # 64
HPC = H // 2         # heads per core = 8
DPC = HPC * DH       # 512 projected dims per core
N_CORES = 8
QC = 512             # q-chunk width
KT = 128             # k-tile width

SCH_A = 184.66496030     # 128 * log2(e)
SCH_B = 16256.0 - 7.4    # 127*128 with mean-centering correction
SCH_MASKED = SCH_B - 1.0e9   # saturates int16 -> -32768 -> bf16 -0.0


def split_excess_waits(nc, cap=1):
    """walrus limits sync-wait slots per ISA instruction (1 for several
    structs).  Move excess waits onto InstEventSemaphore instructions
    inserted just before the offender on the same engine."""
    n_split = 0
    for f in nc.m.functions:
        for blk in f.blocks:
            insts = blk.instructions
            out = []
            changed = False
            for inst in insts:
                si = inst.sync_info
                waits = list(si.on_wait) if si is not None else []
                if len(waits) > cap:
                    for j, w in enumerate(waits[:-cap]):
                        ev = mybir.InstEventSemaphore(
                            name=f"{inst.name}-w{j}", ins=[], outs=[]
                        )
                        ev.engine = inst.engine
                        ev.sync_info = mybir.SyncInfo(on_wait=[w], on_update=[])
                        out.append(ev)
                        n_split += 1
                    inst.sync_info = mybir.SyncInfo(
                        on_wait=waits[-cap:], on_update=list(si.on_update)
                    )
                    changed = True
                out.append(inst)
            if changed:
                blk.instructions = out
    return n_split


INST_LABELS = {}


def build():
    nc = bass.Bass(target_bir_lowering=False)

    _label = ["init"]

    def set_label(s):
        _label[0] = s

    for eng in (nc.tensor, nc.vector, nc.scalar, nc.gpsimd, nc.sync):
        orig = eng.add_instruction

        def wrapped(inst, _orig=orig):
            r = _orig(inst)
            try:
                INST_LABELS[inst.name] = _label[0]
            except Exception:
                pass
            return r

        eng.add_instruction = wrapped

    xT_d = nc.dram_tensor("xT", [D, T], BF16, kind="ExternalInput")
    # host pre-permuted so every per-partition DMA run is contiguous:
    # wqk[p, j, dt, c], wv[p, dt, c], wo[p, hp, c]
    wqk_d = nc.dram_tensor("wqk", [128, 8 * 8 * 128], BF16, kind="ExternalInput")
    wv_d = nc.dram_tensor("wv", [128, 8 * DPC], BF16, kind="ExternalInput")
    wo_d = nc.dram_tensor("wo", [128, 4 * D], BF16, kind="ExternalInput")
    bqk_d = nc.dram_tensor("bqk", [128, 8], F32, kind="ExternalInput")
    bv_d = nc.dram_tensor("bv", [1, DPC], BF16, kind="ExternalInput")
    bmask_d = nc.dram_tensor("bmask", [128, 1024], F32, kind="ExternalInput")
    trimask_d = nc.dram_tensor("trimask", [128, 256], BF16, kind="ExternalInput")
    out_d = nc.dram_tensor("out", [T, D], F32, kind="ExternalOutput")
    rsraw_d = nc.dram_tensor("rsraw", [4, 4096], BF16)  # raw rowsums / chunk
    rrec_d = nc.dram_tensor("rrec", [4, 4096], BF16)    # reciprocals / chunk

    with TileContext(nc) as tc:
        with (
            tc.tile_pool(name="const", bufs=1) as constp,
            tc.tile_pool(name="wstream", bufs=8) as wp,
            tc.tile_pool(name="xt", bufs=2) as xtp,
            tc.tile_pool(name="qk", bufs=1) as qkp,
            tc.tile_pool(name="vaug", bufs=1) as vp,
            tc.tile_pool(name="onorm", bufs=1) as onp,
            tc.tile_pool(name="pt", bufs=4) as ptp,
            tc.tile_pool(name="sc", bufs=8) as scp,
            tc.tile_pool(name="rs", bufs=4) as rsp,
            tc.tile_pool(name="bc", bufs=8) as bcp,
            tc.tile_pool(name="osb", bufs=3) as osbp,
            tc.tile_pool(name="ps", bufs=4, space="PSUM") as psp,
            tc.tile_pool(name="spair", bufs=2, space="PSUM") as spp,
        ):
            # persistent activations
            qk_sb = [
                qkp.tile([128, T], BF16, tag=f"qk{j}", name=f"qk{j}")
                for j in range(8)
            ]
            vaug = [
                vp.tile([128, HPC, DH + 1], BF16, tag=f"v{t}", name=f"v{t}")
                for t in range(16)
            ]
            onorm = [
                onp.tile([128, T], BF16, tag=f"on{hp}", name=f"on{hp}")
                for hp in range(4)
            ]

            set_label("qkv")

            def load_wj(th, j):
                w_j = wp.tile([128, 8, 128], BF16, tag="wqk", name=f"w{th}_{j}")
                wsrc = wqk_d[:, 1024 * j : 1024 * (j + 1)].rearrange(
                    "p (dt c) -> p dt c", c=128
                )
                nc.sync.dma_start(w_j[:], wsrc)
                return w_j

            xts = {}

            def load_x(th):
                t0 = th * (T // 2)
                xt = []
                for dt in range(8):
                    x_t = xtp.tile(
                        [128, T // 2], BF16, tag=f"xt{dt}", name=f"xt{th}_{dt}"
                    )
                    xt.append(x_t)
                for dt in range(8):
                    eng = nc.sync if dt % 2 == 0 else nc.gpsimd
                    eng.dma_start(
                        xt[dt][:],
                        xT_d[128 * dt : 128 * (dt + 1), t0 : t0 + T // 2],
                    )
                xts[th] = xt

            # ---- first loads: w(j=0) + x(th0) before any consts ----
            w_first = load_wj(0, 0)
            load_x(0)

            set_label("const")
            wv_sb = constp.tile([128, 8, DPC], BF16, tag="wv")
            nc.gpsimd.dma_start(
                wv_sb[:], wv_d[:].rearrange("p (dt c) -> p dt c", c=DPC)
            )
            bqk_sb = constp.tile([128, 8], F32, tag="bqk")
            nc.gpsimd.dma_start(bqk_sb[:], bqk_d[:])
            # V bias broadcast to all partitions (DVE add, no bias matmul)
            bvb_sb = constp.tile([128, 8, DH], BF16, tag="bvb")
            nc.gpsimd.dma_start(
                bvb_sb[:].rearrange("p h d -> p (h d)"),
                bass.AP(bv_d, 0, [[0, 128], [1, DPC]]),
            )
            bmask_sb = constp.tile([128, 2, 512], F32, tag="bmask")
            nc.gpsimd.dma_start(
                bmask_sb[:], bmask_d[:].rearrange("p (h q) -> p h q", h=2)
            )
            trimask_sb = constp.tile([128, 2, 128], BF16, tag="trimask")
            nc.gpsimd.dma_start(
                trimask_sb[:], trimask_d[:].rearrange("p (h q) -> p h q", h=2)
            )
            # wo early: oproj chunk 0 runs during t-half-1 projections
            wo_sb = constp.tile([128, 4, D], BF16, tag="wo")
            nc.gpsimd.dma_start(
                wo_sb[:], wo_d[:].rearrange("p (hp c) -> p hp c", c=D)
            )

            # ---- phase 1: projections for one t-half ----
            def proj_half(th):
                t0 = th * (T // 2)
                xt = xts[th]
                set_label("qkv")
                # prefetch the whole W stream upfront (contiguous loads)
                w_tiles = [
                    w_first if (th == 0 and j == 0) else load_wj(th, j)
                    for j in range(8)
                ]
                for j in range(8):
                    w_j = w_tiles[j]
                    ps0 = psp.tile([128, 512], F32, tag="ps", name=f"q{th}{j}a")
                    ps1 = psp.tile([128, 512], F32, tag="ps", name=f"q{th}{j}b")
                    for dt in range(8):
                        # both q-halves share one stationary load
                        nc.tensor.matmul(
                            ps0[:],
                            w_j[:, dt, :],
                            xt[dt][:, 0:512],
                            start=(dt == 0),
                            stop=(dt == 7),
                        )
                        nc.tensor.matmul(
                            ps1[:],
                            w_j[:, dt, :],
                            xt[dt][:, 512:1024],
                            start=(dt == 0),
                            stop=(dt == 7),
                        )
                    nc.vector.tensor_scalar_add(
                        qk_sb[j][:, t0 : t0 + 512], ps0[:], bqk_sb[:, j : j + 1]
                    )
                    nc.vector.tensor_scalar_add(
                        qk_sb[j][:, t0 + 512 : t0 + 1024],
                        ps1[:],
                        bqk_sb[:, j : j + 1],
                    )

                # V (natural layout); bias via DVE broadcast-add; ones col
                set_label("vproj")
                for tt in range(8):
                    tg = th * 8 + tt
                    ps = psp.tile([128, 512], F32, tag="ps", name=f"v{th}{tt}")
                    for dt in range(8):
                        nc.tensor.matmul(
                            ps[:],
                            xt[dt][:, 128 * tt : 128 * (tt + 1)],
                            wv_sb[:, dt, :],
                            start=(dt == 0),
                            stop=(dt == 7),
                        )
                    nc.vector.tensor_tensor(
                        vaug[tg][:, :, 0:DH],
                        ps[:].rearrange("p (h d) -> p h d", h=HPC),
                        bvb_sb[:],
                        ADD,
                    )
                    nc.gpsimd.memset(vaug[tg][:, :, DH : DH + 1], 1.0)

            # ---- phase 2: attention ----
            # exp-engine selection: 'dve' = fused Schraudolph+mask on DVE,
            # 'act' = ScalarE exp (+ GpSimd trimask mult if diagonal)
            def exp_engine(c, j, t):
                if j >= 0:  # diagonal k-tile
                    if c == 0 or (c == 1 and j >= 1):
                        return "act"
                    return "dve"
                # off-diagonal: alternate so each k-tile pair runs one exp
                # on ACT and one on DVE concurrently; bias toward ACT in
                # early chunks where DVE carries the diagonal+norm work
                if t % 2 == 0 and (c == 1 or c == 3 or (c == 2 and t < 6)):
                    return "dve"
                return "act"

            def emit_exp(c, hp, t, sp, pt):
                j = t - 4 * c
                qs = 128 * j if j >= 0 else 0
                eng = exp_engine(c, j, t)
                if j < 0:
                    if eng == "act":
                        nc.scalar.activation(pt[:], sp[:], EXP)
                    else:
                        nc.vector.tensor_scalar(
                            pt[:].bitcast(I16),
                            sp[:],
                            SCH_A,
                            SCH_B,
                            MULT,
                            ADD,
                        )
                    return
                spv = sp[:].rearrange("p (h q) -> p h q", h=2)[:, :, qs:512]
                ptv = pt[:].rearrange("p (h q) -> p h q", h=2)[:, :, qs:512]
                if eng == "dve":
                    nc.vector.scalar_tensor_tensor(
                        ptv.bitcast(I16),
                        spv,
                        SCH_A,
                        bmask_sb[:, :, 0 : 512 - qs],
                        MULT,
                        ADD,
                    )
                else:
                    nc.scalar.activation(ptv, spv, EXP)
                    blk = pt[:].rearrange("p (h q) -> p h q", h=2)[
                        :, :, qs : qs + 128
                    ]
                    nc.gpsimd.tensor_tensor(blk, blk, trimask_sb[:], MULT)

            def attn_block(c, hp):
                set_label("attn")
                qT = qk_sb[hp]
                kT = qk_sb[4 + hp]
                q0 = QC * c
                ktiles = 4 * (c + 1)
                oA = psp.tile([128, 512], F32, tag="ps", name=f"oA{c}{hp}")
                oB = psp.tile([128, 512], F32, tag="ps", name=f"oB{c}{hp}")

                def s_mm(t):
                    j = t - 4 * c
                    qs = 128 * j if j >= 0 else 0
                    sp = spp.tile([128, 1024], F32, tag="sp", name=f"sp{t}")
                    for half, base in ((0, 0), (1, 64)):
                        nc.tensor.matmul(
                            sp[:, 512 * half + qs : 512 * (half + 1)],
                            kT[base : base + 64, 128 * t : 128 * (t + 1)],
                            qT[base : base + 64, q0 + qs : q0 + QC],
                            start=True,
                            stop=True,
                            tile_position=(base, 0),
                        )
                    return sp

                def av_mm(t, pt):
                    j = t - 4 * c
                    qs = 128 * j if j >= 0 else 0
                    for o_ps, half in ((oA, 0), (oB, 1)):
                        nc.tensor.matmul(
                            o_ps[0 : DH + 1, qs:512],
                            vaug[t][:, 2 * hp + half, :],
                            pt[:, 512 * half + qs : 512 * (half + 1)],
                            start=(t == 0),
                            stop=(t == ktiles - 1),
                            skip_group_check=True,
                        )

                # k-tiles in pairs: S,S -> exp,exp -> AV,AV
                t = 0
                while t < ktiles:
                    pair = [t] if t + 1 >= ktiles else [t, t + 1]
                    sps, pts = [], []
                    for u in pair:
                        sps.append(s_mm(u))
                    for i, u in enumerate(pair):
                        pt = ptp.tile([128, 1024], BF16, tag="pt", name=f"pt{u}")
                        emit_exp(c, hp, u, sps[i], pt)
                        pts.append(pt)
                    for i, u in enumerate(pair):
                        av_mm(u, pts[i])
                    t += len(pair)

                # evict raw o + rowsums from psum; all SBUF staging tiles
                # at partition base 0
                set_label("norm")
                sch = []
                for o_ps, half in ((oA, 0), (oB, 1)):
                    idx = 2 * hp + half
                    rrow = rsp.tile([1, 512], BF16, tag="rrow")
                    nc.vector.tensor_copy(
                        out=rrow[:], in_=o_ps[DH : DH + 1, :]
                    )
                    nc.gpsimd.dma_start(
                        rsraw_d[c : c + 1, idx * 512 : (idx + 1) * 512],
                        rrow[:],
                    )
                    sc = scp.tile([64, 512], BF16, tag="sc", name=f"sc{half}")
                    nc.vector.tensor_copy(out=sc[:], in_=o_ps[0:DH, :])
                    sch.append(sc)
                return sch

            def attn_chunk(c):
                scs = []
                for hp in range(4):
                    scs.append(attn_block(c, hp))
                # batched reciprocal: reload the chunk's 8 rowsum rows
                # respread as [128, 32] so the per-lane-serial reciprocal
                # touches only 32 elements per lane
                set_label("norm")
                rload = rsp.tile([128, 32], BF16, tag="rload")
                nc.sync.dma_start(
                    rload[:], bass.AP(rsraw_d, c * 4096, [[32, 128], [1, 32]])
                )
                rrec = rsp.tile([128, 32], BF16, tag="rrec")
                with nc.allow_low_precision(
                    reason="bf16 softmax-normalizer reciprocal; 0.4% rel "
                    "is far inside the output tolerance"
                ):
                    nc.vector.reciprocal(rrec[:], rload[:])
                nc.sync.dma_start(
                    bass.AP(rrec_d, c * 4096, [[32, 128], [1, 32]]), rrec[:]
                )
                for hp in range(4):
                    for half in range(2):
                        bc = bcp.tile([64, 512], BF16, tag="bc")
                        nc.sync.dma_start(
                            bc[:],
                            bass.AP(
                                rrec_d,
                                c * 4096 + (2 * hp + half) * 512,
                                [[0, 64], [1, 512]],
                            ),
                        )
                        nc.vector.tensor_tensor(
                            onorm[hp][64 * half : 64 * half + 64, QC * c : QC * (c + 1)],
                            scs[hp][half][:],
                            bc[:],
                            MULT,
                        )

            # ---- phase 3: output projection for one q-chunk ----
            def oproj_chunk(c):
                set_label("oproj")
                for qt in range(4 * c, 4 * c + 4):
                    ps0 = psp.tile([128, 512], F32, tag="ps", name=f"o{qt}a")
                    ps1 = psp.tile([128, 512], F32, tag="ps", name=f"o{qt}b")
                    for hp in range(4):
                        # both d-halves share one stationary load
                        nc.tensor.matmul(
                            ps0[:],
                            onorm[hp][:, 128 * qt : 128 * (qt + 1)],
                            wo_sb[:, hp, 0:512],
                            start=(hp == 0),
                            stop=(hp == 3),
                        )
                        nc.tensor.matmul(
                            ps1[:],
                            onorm[hp][:, 128 * qt : 128 * (qt + 1)],
                            wo_sb[:, hp, 512:1024],
                            start=(hp == 0),
                            stop=(hp == 3),
                        )
                    for dc, ps in ((0, ps0), (1, ps1)):
                        osb = osbp.tile([128, 512], F32, tag="osb")
                        nc.scalar.activation(osb[:], ps[:], COPY)
                        nc.gpsimd.dma_start(
                            out_d[
                                128 * qt : 128 * (qt + 1),
                                512 * dc : 512 * (dc + 1),
                            ],
                            osb[:],
                        )

            # ---- emission order ----
            proj_half(0)
            load_x(1)          # prefetch t-half-1 x during early attention
            attn_chunk(0)
            attn_chunk(1)
            oproj_chunk(0)
            proj_half(1)
            oproj_chunk(1)
            attn_chunk(3)      # big chunk first: its oproj hides under a2
            attn_chunk(2)
            oproj_chunk(3)
            oproj_chunk(2)

    split_excess_waits(nc)
    return nc


TRACE = False
LAST_EXEC_NS = None

_NC = None


def _get_nc():
    global _NC
    if _NC is None:
        _NC = build()
    return _NC


def kernel(x, W_qkv, b_qkv, W_o, b_o):
    x = np.asarray(x, dtype=np.float32)
    W_qkv = np.asarray(W_qkv, dtype=np.float32)
    b_qkv = np.asarray(b_qkv, dtype=np.float32)
    W_o = np.asarray(W_o, dtype=np.float32)
    b_o = np.asarray(b_o, dtype=np.float32)
    import ml_dtypes

    BF = ml_dtypes.bfloat16
    scale = 1.0 / np.sqrt(np.float32(DH))  # = 0.125, exact in bf16

    # x^T per batch (shared between the two cores of a batch)
    xTs = [np.ascontiguousarray(x[b].T.astype(BF)) for b in range(B)]

    # Bmask [128, 2 heads, 512]: for the k-tile suffix starting at the
    # diagonal block: first 128 cols triangular (keep iff q >= k), rest keep.
    bm1 = np.full((128, 512), np.float32(SCH_B), np.float32)
    qq = np.arange(128)[None, :]
    pp = np.arange(128)[:, None]
    blk = np.where(qq >= pp, np.float32(SCH_B), np.float32(SCH_MASKED))
    bm1[:, 0:128] = blk
    bmask = np.concatenate([bm1, bm1], axis=1).astype(np.float32)

    # 0/1 triangular mask for the ScalarE-exp diagonal path
    tri1 = np.triu(np.ones((128, 128), np.float32))
    trimask = np.concatenate([tri1, tri1], axis=1).astype(BF)

    in_maps = []
    for c in range(N_CORES):
        b, g = divmod(c, 2)
        h0 = g * HPC
        qcols = slice(h0 * DH, h0 * DH + DPC)
        kcols = slice(D + h0 * DH, D + h0 * DH + DPC)
        vcols = slice(2 * D + h0 * DH, 2 * D + h0 * DH + DPC)
        wqk = np.concatenate(
            [W_qkv[:, qcols] * scale, W_qkv[:, kcols]], axis=1
        ).astype(BF)
        # pre-permute weights so each per-partition DMA run is contiguous
        wqk_hw = wqk.reshape(8, 128, 8, 128).transpose(1, 2, 0, 3).reshape(128, 8192)
        wv_hw = (
            W_qkv[:, vcols]
            .astype(BF)
            .reshape(8, 128, DPC)
            .transpose(1, 0, 2)
            .reshape(128, 8 * DPC)
        )
        wo_hw = (
            W_o[g * DPC : (g + 1) * DPC, :]
            .astype(BF)
            .reshape(4, 128, D)
            .transpose(1, 0, 2)
            .reshape(128, 4 * D)
        )
        bqk = np.concatenate(
            [b_qkv[qcols] * scale, b_qkv[kcols]]
        ).astype(np.float32)
        in_maps.append(
            {
                "xT": xTs[b],
                "wqk": np.ascontiguousarray(wqk_hw),
                "wv": np.ascontiguousarray(wv_hw),
                "wo": np.ascontiguousarray(wo_hw),
                "bqk": np.ascontiguousarray(bqk.reshape(8, 128).T),
                "bv": np.ascontiguousarray(b_qkv[vcols].astype(BF).reshape(1, DPC)),
                "bmask": bmask,
                "trimask": trimask,
            }
        )

    nc = _get_nc()
    global LAST_EXEC_NS
    res = None
    last_err = None
    for attempt in range(3):
        try:
            res = run_bass_kernel_spmd(
                nc, in_maps, list(range(N_CORES)), trace=TRACE
            )
            break
        except Exception as e:  # transient device wedge: retry
            last_err = e
            import time as _time

            _time.sleep(5)
    if res is None:
        raise last_err
    LAST_EXEC_NS = res.exec_time_ns
    globals()["_LAST_RES"] = res
    parts = [res.results[c]["out"] for c in range(N_CORES)]
    out = np.empty((B, T, D), np.float32)
    for b in range(B):
        out[b] = parts[2 * b] + parts[2 * b + 1] + b_o[None, :]
    return out
